# revision 6
# baseline (speedup 1.0000x reference)
"""Trainium2 Bass kernel for 4-head spatial attention score softmax.

Reference computation:
    qk = einsum('bcxy,oc->boxy', fmap[1,256,64,64], W_qk[1024,256])
    q, k = split(qk, 2, axis=1)             # each [1, 512, 64, 64]
    q = q reshaped to heads, scaled by 128^-0.5
    sim[b,h,xy,uv] = q . k  (contraction over dim_head=128)
    out = softmax(sim, axis=-1)             # [1, 4, 4096, 4096] f32
    (b=1, 4096 = 64*64 spatial positions)

Sharding: 8 cores = 4 heads x 2 query-halves. Each core projects q for its
2048 query columns + k for all 4096 columns (PE matmuls over the channel
dim), computes scores with fp16 matmuls, softmax (exp on ScalarE with
accumulated row sums, normalize on VectorE), and streams its [2048, 4096]
f32 output slab to HBM.

The run is DMA-bound: the 33.55MB output write streams at the ~360-420GB/s
per-core bus cap with zero gaps, so total time = prologue + stream. The
prologue is engineered around the trace-observed constraints (PE executes
in order at ~215ns per 512-wide fp16 matmul when hot; the HAM governor
halves the PE clock ~3.4us after any idle window; PSUM->SBUF copies cost
~1.2ns/elem on either VectorE or ScalarE):
  - fmap + weights are sent as fp16, with fmap packed on host so each
    1024-column load chunk is 4KB-contiguous per partition (one DMA
    descriptor per partition per chunk).
  - fmap columns are pre-rotated per core on host so this core's query
    columns are always columns [0, 2048): the q projection uses static
    offsets and depends only on the first two load chunks. The host
    un-rotates the output columns during assembly (a half-swap).
  - input loads are split across the Scalar and Vector DMA queues (and
    away from the Sync queue the stores use, so the first store is not
    queued behind the load in ring order).
  - projections are chunked 1:1 with the load chunks, emitted in
    dependency order with no filler in between; a single warmup burst
    sized to end near chunk 0's arrival keeps the PE clock hot.
  - PSUM->SBUF copies are split across VectorE (k) and ScalarE (q);
    tile 0's two normalizes also run on both engines in parallel.
"""

import numpy as np

import concourse.bacc as bacc
import concourse.mybir as mybir
import concourse.tile as tile
from concourse import bass_utils

HEADS = 4
DIM_HEAD = 128
C = 256          # input channels
XY = 4096        # 64*64 spatial positions
QCHUNK = 2048    # query positions per core
N_CORES = 8
SCALE = DIM_HEAD ** -0.5

F32 = mybir.dt.float32

# dtype of everything the PE touches (fmap, weights, q, k). 16-bit halves
# both the HBM load bytes and the PE streaming cost vs f32r, and enables
# fast weight load. fp16 over bf16: all values are O(1), so the e5m10
# mantissa (exact inside the PE's FP22) cuts quantization error ~8x.
# NOTE: both matmul operands MUST share one dtype - mixing fp16/bf16 in a
# single matmul hard-crashes the device (NRT_EXEC_UNIT_UNRECOVERABLE).
QK_DT = mybir.dt.float16

KCH = 1024       # load/projection chunk width (columns)
NCH = XY // KCH  # 4 chunks


def _emit(tc, fmap_p, wqkt, out):
    nc = tc.nc

    with tc.tile_pool(name="consts", bufs=1) as consts:
        w_sb = consts.tile([128, 2, 2 * DIM_HEAD], QK_DT)
        # fmap chunk g: [128p, a, col] lives at fk_sb[:, g]
        fk_sb = consts.tile([128, NCH, 2, KCH], QK_DT)
        warm_sb = consts.tile([128, 512], QK_DT)
        q_sb = consts.tile([128, QCHUNK], QK_DT)  # [d, x] for this core's queries
        k_sb = consts.tile([128, XY], QK_DT)      # [d, uv]

        # memset first so the PE warmup isn't stuck behind DVE's dma issues
        nc.vector.memset(warm_sb, 0.0)

        # Input loads split across the Scalar + Vector DMA queues: two
        # descriptor rings fill the DMA engines faster, and the Sync
        # queue's ring stays empty for the first output store.
        # wqkt = [wq.T | wk.T] concatenated on host: [c, d] with c split
        # into 2 partition chunks.
        nc.gpsimd.dma_start(out=w_sb, in_=wqkt.rearrange("(a p) d -> p a d", p=128))
        for g in range(NCH):
            eng = nc.scalar if g % 2 == 0 else nc.gpsimd
            eng.dma_start(out=fk_sb[:, g],
                          in_=fmap_p[:, g * 2 * KCH:(g + 1) * 2 * KCH])

        # One PSUM pool + tag for warmup, projections, and scores: the
        # natural 2-slot rotation makes each new tile wait only on the
        # previous-but-one tile's consumer.
        with tc.tile_pool(name="ps", bufs=2, space="PSUM") as ps_pool, \
             tc.tile_pool(name="soft", bufs=6) as soft_pool, \
             tc.tile_pool(name="small", bufs=4) as small_pool:
            # PE warmup: dummy matmuls with no load deps ramp the HAM
            # clock gate to 2.4 GHz; sized to end near chunk 0's arrival
            # (PE is in-order, so oversizing delays the projections).
            warm_ps = ps_pool.tile([128, 2048], F32, tag="ps")
            for i in range(7):
                nc.tensor.matmul(warm_ps[:, 0:512], lhsT=warm_sb[:, 0:128],
                                 rhs=warm_sb, start=True, stop=True)

            # ---- per-chunk projections: out[d, n] = sum_c W^T[c, d] * fmap[c, n]
            def emit_proj(g, which):
                dlo = 0 if which == "q" else DIM_HEAD
                ps_p = ps_pool.tile([128, 2048], F32, tag="ps",
                                    name=f"ps_{which}{g}")
                for j in range(KCH // 512):
                    osl = slice(j * 512, (j + 1) * 512)
                    nc.tensor.matmul(ps_p[:, osl],
                                     lhsT=w_sb[:, 0, dlo:dlo + DIM_HEAD],
                                     rhs=fk_sb[:, g, 0, j * 512:(j + 1) * 512],
                                     start=True, stop=False)
                    nc.tensor.matmul(ps_p[:, osl],
                                     lhsT=w_sb[:, 1, dlo:dlo + DIM_HEAD],
                                     rhs=fk_sb[:, g, 1, j * 512:(j + 1) * 512],
                                     start=False, stop=True)
                dst = q_sb if which == "q" else k_sb
                # k copies on VectorE, q copies on ScalarE: the two copy
                # streams run in parallel instead of serializing on DVE.
                if which == "q":
                    nc.scalar.copy(dst[:, g * KCH:(g + 1) * KCH],
                                   ps_p[:, 0:KCH])
                else:
                    nc.vector.tensor_copy(dst[:, g * KCH:(g + 1) * KCH],
                                          ps_p[:, 0:KCH])

            # Dependency-ordered: chunk g's projections unblock when load
            # chunk g lands; q (= columns [0, 2048)) comes from chunks 0-1.
            emit_proj(0, "k")
            emit_proj(0, "q")
            emit_proj(1, "k")
            emit_proj(1, "q")
            emit_proj(2, "k")
            emit_proj(3, "k")

            # ---- scores + softmax, 16 query tiles of 128 ----
            for qt in range(QCHUNK // 128):
                qsl = q_sb[:, qt * 128:(qt + 1) * 128]
                et = soft_pool.tile([128, XY], F32, tag="et")
                # Tile 0 splits the exp into 1024-wide chunks so the first
                # store only waits on the last k chunk's 512-wide matmuls,
                # not a whole 2048-wide exp. Steady-state tiles use the
                # cheaper 2-instruction exp.
                nexp = 4 if qt == 0 else 2
                ech = XY // nexp
                pp = small_pool.tile([128, 4], F32, tag="pp")
                for half in range(2):
                    ps = ps_pool.tile([128, 2048], F32, tag="ps")
                    for j in range(4):
                        osl = slice(j * 512, (j + 1) * 512)
                        ksl = slice(half * 2048 + j * 512, half * 2048 + (j + 1) * 512)
                        nc.tensor.matmul(ps[:, osl], lhsT=qsl,
                                         rhs=k_sb[:, ksl],
                                         start=True, stop=True)
                    # exp straight out of PSUM, with per-row partial sums
                    # accumulated for free.
                    for e in range(nexp // 2):
                        psl = slice(e * ech, (e + 1) * ech)
                        idx = half * (nexp // 2) + e
                        nc.scalar.activation(
                            out=et[:, half * 2048 + e * ech:
                                   half * 2048 + (e + 1) * ech],
                            in_=ps[:, psl],
                            func=mybir.ActivationFunctionType.Exp,
                            accum_out=pp[:, idx:idx + 1])
                den = small_pool.tile([128, 1], F32, tag="den")
                if nexp == 2:
                    nc.vector.tensor_add(den, pp[:, 0:1], pp[:, 1:2])
                else:
                    nc.vector.tensor_reduce(den, pp[:, 0:nexp],
                                            axis=mybir.AxisListType.X,
                                            op=mybir.AluOpType.add)
                nc.vector.reciprocal(den, den)
                if qt == 0:
                    # normalize + store in halves, the two normalizes on
                    # different engines: the first bytes hit HBM sooner.
                    nc.vector.tensor_scalar_mul(et[:, 0:2048], et[:, 0:2048], den)
                    nc.sync.dma_start(out=out[qt * 128:(qt + 1) * 128, 0:2048],
                                      in_=et[:, 0:2048])
                    nc.scalar.mul(et[:, 2048:4096], et[:, 2048:4096], den)
                    nc.sync.dma_start(out=out[qt * 128:(qt + 1) * 128, 2048:4096],
                                      in_=et[:, 2048:4096])
                else:
                    nc.vector.tensor_scalar_mul(et, et, den)
                    nc.sync.dma_start(out=out[qt * 128:(qt + 1) * 128, :],
                                      in_=et)


def build_program():
    nc = bacc.Bacc("TRN2", target_bir_lowering=False, debug=False,
                   enable_asserts=False)
    fmap_p = nc.dram_tensor("fmap_p", [128, 2 * XY], QK_DT,
                            kind="ExternalInput").ap()
    wqkt = nc.dram_tensor("wqkt", [C, 2 * DIM_HEAD], QK_DT,
                          kind="ExternalInput").ap()
    out = nc.dram_tensor("out", [QCHUNK, XY], F32, kind="ExternalOutput").ap()

    with tile.TileContext(nc) as tc:
        _emit(tc, fmap_p, wqkt, out)
    nc.compile()
    return nc


_CACHE = {}


def _get_nc():
    if "nc" not in _CACHE:
        _CACHE["nc"] = build_program()
    return _CACHE["nc"]


def _pack_fmap(fm):
    """[256, 4096] fp16 -> [128, 8192] with layout [p][chunk][a][col]:
    partition p holds channels p (a=0) and 128+p (a=1); each 1024-column
    chunk is 4KB-contiguous per partition."""
    t = fm.reshape(2, 128, NCH, KCH)                  # [a, p, g, c]
    return np.ascontiguousarray(
        t.transpose(1, 2, 0, 3).reshape(128, 2 * XY))  # [p][g][a][c]


def make_in_maps(fmap, W_qk):
    fm = np.asarray(fmap, dtype=np.float32).reshape(C, XY)
    # Core 2h+1 sees fmap columns rotated left by QCHUNK, so its query
    # columns sit at [0, 2048) like core 2h's. 4096-col rotation by 2048
    # == swapping the two column halves.
    fm_rot = np.concatenate([fm[:, QCHUNK:], fm[:, :QCHUNK]],
                            axis=1).astype(np.float16)
    fm = fm.astype(np.float16)
    packed = _pack_fmap(fm)
    packed_rot = _pack_fmap(fm_rot)
    W = np.asarray(W_qk, dtype=np.float32)
    in_maps = []
    for core in range(N_CORES):
        hd, qhalf = divmod(core, 2)
        wq = W[hd * DIM_HEAD:(hd + 1) * DIM_HEAD] * np.float32(SCALE)
        wk = W[HEADS * DIM_HEAD + hd * DIM_HEAD:
               HEADS * DIM_HEAD + (hd + 1) * DIM_HEAD]
        in_maps.append({
            "fmap_p": packed_rot if qhalf else packed,
            "wqkt": np.concatenate([wq.T, wk.T], axis=1).astype(np.float16),
        })
    return in_maps


def assemble(per_core_outs):
    out = np.empty((HEADS, XY, XY), dtype=np.float32)
    for core in range(N_CORES):
        hd, qhalf = divmod(core, 2)
        slab = per_core_outs[core]
        rows = slice(qhalf * QCHUNK, (qhalf + 1) * QCHUNK)
        if qhalf:
            # un-rotate the uv columns (slab col j = true col (j+2048)%4096)
            out[hd, rows, :QCHUNK] = slab[:, QCHUNK:]
            out[hd, rows, QCHUNK:] = slab[:, :QCHUNK]
        else:
            out[hd, rows, :] = slab
    return out.reshape(1, HEADS, XY, XY)


def kernel(fmap, W_qk, trace=False):
    nc = _get_nc()
    in_maps = make_in_maps(fmap, W_qk)
    res = bass_utils.run_bass_kernel_spmd(
        nc, in_maps, core_ids=list(range(N_CORES)), trace=trace)
    out = assemble([res.results[c]["out"] for c in range(N_CORES)])
    if trace:
        kernel.last_exec_time_ns = res.exec_time_ns
        kernel.last_results = res
    return out


# revision 7
# speedup vs baseline: 1.0021x; 1.0021x over previous
"""Trainium2 Bass kernel for 4-head spatial attention score softmax.

Reference computation:
    qk = einsum('bcxy,oc->boxy', fmap[1,256,64,64], W_qk[1024,256])
    q, k = split(qk, 2, axis=1)             # each [1, 512, 64, 64]
    q = q reshaped to heads, scaled by 128^-0.5
    sim[b,h,xy,uv] = q . k  (contraction over dim_head=128)
    out = softmax(sim, axis=-1)             # [1, 4, 4096, 4096] f32
    (b=1, 4096 = 64*64 spatial positions)

Sharding: 8 cores = 4 heads x 2 query-halves. Each core projects q for its
2048 query columns + k for all 4096 columns (PE matmuls over the channel
dim), computes scores with fp16 matmuls, softmax (exp on ScalarE with
accumulated row sums, normalize on VectorE), and streams its [2048, 4096]
f32 output slab to HBM.

The run is DMA-bound: the 33.55MB output write streams at the ~360-424GB/s
per-core cap (all 16 DMA engines 99% busy) with zero gaps, so total time =
first-store-time + stream. The prologue is engineered around the
trace-observed constraints:
  - PE executes in order at ~215ns per 512-wide fp16 matmul when hot; the
    HAM governor halves the clock ~3.4us after any PE idle window, so the
    warmup burst is sized to end right as the first fmap chunk lands and
    the projections are emitted in chunk-arrival order with no filler.
  - a single load queue sustains only ~170GB/s for 4KB read descriptors,
    so the load is split across all three DMA-capable queues (Scalar,
    Sync, GpSimd). The Sync ring drains ~10us before the first store is
    enqueued on it. fmap is packed on host so every chunk is contiguous
    per partition, and pre-rotated per core so this core's query columns
    are always [0, 2048) (static offsets; the host un-rotates the output
    columns during assembly, a half-swap).
  - PSUM->SBUF copies: k casts on VectorE, q copies on ScalarE, so the
    two copy streams run in parallel and ScalarE stays free for the exp
    chain right after the first score matmuls.
  - tile 0 normalizes + stores in 1024-col quarters (first bytes to HBM
    ~0.6us after the row sum), tile 1 in halves; steady-state tiles do
    one full-row normalize + store, which the stream cadence hides.
"""

import numpy as np

import concourse.bacc as bacc
import concourse.mybir as mybir
import concourse.tile as tile
from concourse import bass_utils

HEADS = 4
DIM_HEAD = 128
C = 256          # input channels
XY = 4096        # 64*64 spatial positions
QCHUNK = 2048    # query positions per core
N_CORES = 8
SCALE = DIM_HEAD ** -0.5

F32 = mybir.dt.float32

# dtype of everything the PE touches (fmap, weights, q, k). 16-bit halves
# both the HBM load bytes and the PE streaming cost vs f32r, and enables
# fast weight load. fp16 over bf16: all values are O(1), so the e5m10
# mantissa (exact inside the PE's FP22) cuts quantization error ~8x.
# NOTE: both matmul operands MUST share one dtype - mixing fp16/bf16 in a
# single matmul hard-crashes the device (NRT_EXEC_UNIT_UNRECOVERABLE).
QK_DT = mybir.dt.float16

# load chunks: 4x512 columns over the q region, then 2x1024 for the k tail
CHUNKS = [512, 512, 512, 512, 1024, 1024]
CHUNK_OFF = [0, 512, 1024, 1536, 2048, 3072]


def _emit(tc, fmap_p, wqkt, out):
    nc = tc.nc

    with tc.tile_pool(name="consts", bufs=1) as consts:
        w_sb = consts.tile([128, 2, 2 * DIM_HEAD], QK_DT)
        # fmap column chunk g: [128p, a, col] at fk_sb[:, 2*off : 2*(off+w)]
        fk_sb = consts.tile([128, 2 * XY], QK_DT)
        warm_sb = consts.tile([128, 512], QK_DT)
        q_sb = consts.tile([128, QCHUNK], QK_DT)  # [d, x] for this core's queries
        k_sb = consts.tile([128, XY], QK_DT)      # [d, uv]

        # memset first so the PE warmup isn't stuck behind dma issues
        nc.vector.memset(warm_sb, 0.0)

        # Input loads fanned across the three DMA-capable queues (one
        # ring sustains only ~170GB/s on 2-4KB read descriptors). The
        # weights go first on the Scalar ring (needed by the first
        # projection); the GpSimd SWDGE ring starts ~1.5us late, so it
        # gets chunks that are needed later.
        queues = [nc.scalar, nc.sync, nc.gpsimd]
        nc.scalar.dma_start(out=w_sb,
                            in_=wqkt.rearrange("(a p) d -> p a d", p=128))
        for g, (off, width) in enumerate(zip(CHUNK_OFF, CHUNKS)):
            eng = queues[[0, 1, 2, 0, 1, 2][g]]
            eng.dma_start(out=fk_sb[:, 2 * off:2 * (off + width)],
                          in_=fmap_p[:, 2 * off:2 * (off + width)])

        # One PSUM pool + tag for warmup, projections, and scores: the
        # 2-slot rotation makes each new tile wait only on the
        # previous-but-one tile's consumer.
        with tc.tile_pool(name="ps", bufs=2, space="PSUM") as ps_pool, \
             tc.tile_pool(name="soft", bufs=6) as soft_pool, \
             tc.tile_pool(name="small", bufs=4) as small_pool:
            # PE warmup: dummy matmuls with no load deps ramp the HAM
            # clock gate to 2.4 GHz; sized to end near chunk 0's arrival
            # (PE is in-order, so oversizing delays the projections).
            warm_ps = ps_pool.tile([128, 2048], F32, tag="ps")
            for i in range(5):
                nc.tensor.matmul(warm_ps[:, 0:512], lhsT=warm_sb[:, 0:128],
                                 rhs=warm_sb, start=True, stop=True)

            # ---- per-chunk projections: out[d, n] = sum_c W^T[c, d] * fmap[c, n]
            def emit_proj(g, which):
                off, width = CHUNK_OFF[g], CHUNKS[g]
                dlo = 0 if which == "q" else DIM_HEAD
                ps_p = ps_pool.tile([128, 2048], F32, tag="ps",
                                    name=f"ps_{which}{g}")
                for j in range(width // 512):
                    osl = slice(j * 512, (j + 1) * 512)
                    for a in range(2):
                        fsl = slice(2 * off + a * width + j * 512,
                                    2 * off + a * width + (j + 1) * 512)
                        nc.tensor.matmul(ps_p[:, osl],
                                         lhsT=w_sb[:, a, dlo:dlo + DIM_HEAD],
                                         rhs=fk_sb[:, fsl],
                                         start=(a == 0), stop=(a == 1))
                # k casts on VectorE, q copies on ScalarE: the two copy
                # streams run in parallel instead of serializing.
                if which == "q":
                    nc.scalar.copy(q_sb[:, off:off + width], ps_p[:, 0:width])
                else:
                    nc.vector.tensor_copy(k_sb[:, off:off + width],
                                          ps_p[:, 0:width])

            # Dependency-ordered: chunk g's projections unblock when load
            # chunk g lands; q (= columns [0, 2048)) comes from chunks 0-3.
            for g in range(4):
                emit_proj(g, "k")
                emit_proj(g, "q")
            emit_proj(4, "k")
            emit_proj(5, "k")

            # ---- scores + softmax, 16 query tiles of 128 ----
            for qt in range(QCHUNK // 128):
                qsl = q_sb[:, qt * 128:(qt + 1) * 128]
                et = soft_pool.tile([128, XY], F32, tag="et")
                pp = small_pool.tile([128, 2], F32, tag="pp")
                for half in range(2):
                    ps = ps_pool.tile([128, 2048], F32, tag="ps")
                    for j in range(4):
                        osl = slice(j * 512, (j + 1) * 512)
                        ksl = slice(half * 2048 + j * 512,
                                    half * 2048 + (j + 1) * 512)
                        nc.tensor.matmul(ps[:, osl], lhsT=qsl,
                                         rhs=k_sb[:, ksl],
                                         start=True, stop=True)
                    # exp straight out of PSUM, with per-row partial sums
                    # accumulated for free.
                    nc.scalar.activation(
                        out=et[:, half * 2048:(half + 1) * 2048],
                        in_=ps,
                        func=mybir.ActivationFunctionType.Exp,
                        accum_out=pp[:, half:half + 1])
                den = small_pool.tile([128, 1], F32, tag="den")
                nc.vector.tensor_add(den, pp[:, 0:1], pp[:, 1:2])
                nc.vector.reciprocal(den, den)
                # Early tiles normalize + store in column pieces so the
                # first bytes hit HBM sooner; afterwards the stream is
                # saturated and full-row stores are cheapest.
                npieces = 4 if qt == 0 else (2 if qt == 1 else 1)
                pw = XY // npieces
                for p2 in range(npieces):
                    sl2 = slice(p2 * pw, (p2 + 1) * pw)
                    nc.vector.tensor_scalar_mul(et[:, sl2], et[:, sl2], den)
                    nc.sync.dma_start(out=out[qt * 128:(qt + 1) * 128, sl2],
                                      in_=et[:, sl2])


def build_program():
    nc = bacc.Bacc("TRN2", target_bir_lowering=False, debug=False,
                   enable_asserts=False)
    fmap_p = nc.dram_tensor("fmap_p", [128, 2 * XY], QK_DT,
                            kind="ExternalInput").ap()
    wqkt = nc.dram_tensor("wqkt", [C, 2 * DIM_HEAD], QK_DT,
                          kind="ExternalInput").ap()
    out = nc.dram_tensor("out", [QCHUNK, XY], F32, kind="ExternalOutput").ap()

    with tile.TileContext(nc) as tc:
        _emit(tc, fmap_p, wqkt, out)
    nc.compile()
    return nc


_CACHE = {}


def _get_nc():
    if "nc" not in _CACHE:
        _CACHE["nc"] = build_program()
    return _CACHE["nc"]


def _pack_fmap(fm):
    """[256, 4096] fp16 -> [128, 8192] where partition p holds channels p
    (a=0) and 128+p (a=1), and each load chunk [off, off+width) is
    contiguous per partition as [a=0 cols | a=1 cols]."""
    t = fm.reshape(2, 128, XY)                        # [a, p, n]
    packed = np.empty((128, 2 * XY), dtype=np.float16)
    for off, width in zip(CHUNK_OFF, CHUNKS):
        seg = t[:, :, off:off + width]                # [a, p, w]
        packed[:, 2 * off:2 * (off + width)] = (
            seg.transpose(1, 0, 2).reshape(128, 2 * width))
    return packed


def make_in_maps(fmap, W_qk):
    fm = np.asarray(fmap, dtype=np.float32).reshape(C, XY)
    # Core 2h+1 sees fmap columns rotated left by QCHUNK, so its query
    # columns sit at [0, 2048) like core 2h's. 4096-col rotation by 2048
    # == swapping the two column halves.
    fm_rot = np.concatenate([fm[:, QCHUNK:], fm[:, :QCHUNK]],
                            axis=1).astype(np.float16)
    fm = fm.astype(np.float16)
    packed = _pack_fmap(fm)
    packed_rot = _pack_fmap(fm_rot)
    W = np.asarray(W_qk, dtype=np.float32)
    in_maps = []
    for core in range(N_CORES):
        hd, qhalf = divmod(core, 2)
        wq = W[hd * DIM_HEAD:(hd + 1) * DIM_HEAD] * np.float32(SCALE)
        wk = W[HEADS * DIM_HEAD + hd * DIM_HEAD:
               HEADS * DIM_HEAD + (hd + 1) * DIM_HEAD]
        in_maps.append({
            "fmap_p": packed_rot if qhalf else packed,
            "wqkt": np.concatenate([wq.T, wk.T], axis=1).astype(np.float16),
        })
    return in_maps


def assemble(per_core_outs):
    out = np.empty((HEADS, XY, XY), dtype=np.float32)
    for core in range(N_CORES):
        hd, qhalf = divmod(core, 2)
        slab = per_core_outs[core]
        rows = slice(qhalf * QCHUNK, (qhalf + 1) * QCHUNK)
        if qhalf:
            # un-rotate the uv columns (slab col j = true col (j+2048)%4096)
            out[hd, rows, :QCHUNK] = slab[:, QCHUNK:]
            out[hd, rows, QCHUNK:] = slab[:, :QCHUNK]
        else:
            out[hd, rows, :] = slab
    return out.reshape(1, HEADS, XY, XY)


def kernel(fmap, W_qk, trace=False):
    nc = _get_nc()
    in_maps = make_in_maps(fmap, W_qk)
    res = bass_utils.run_bass_kernel_spmd(
        nc, in_maps, core_ids=list(range(N_CORES)), trace=trace)
    out = assemble([res.results[c]["out"] for c in range(N_CORES)])
    if trace:
        kernel.last_exec_time_ns = res.exec_time_ns
        kernel.last_results = res
    return out
